# revision 4
# baseline (speedup 1.0000x reference)
"""DeepOHeat-ST rank-contraction kernel for 8x Trainium2 NeuronCores.

Computes einsum('iyz,jyz,kyz,lyz,byz->bijkly', t1,t2,t3,t4,b) where the
t_i / b factors come from tiny Fourier-feature MLP trunks and a branch MLP.

Sharding: data-parallel over the j axis (trunk-2 evaluation points).  Each
of the 8 cores receives its 8 rows of x2, evaluates all MLPs (replicated,
~1% of the FLOPs), builds
    U^T[z, b*64+i] = bT[z,b] * t1T[z,i]                    (512 cols)
    W_p[z, (jl*64+k)*32+l] = t2T[z,jl]*t3T[z,k]*t4T[z,l]   (4x 4096 cols)
and evaluates its 512x16384 output shard as K=64 (zero-padded to 128)
bf16 matmuls (N=512), PSUM -> SBUF staging -> 2MB DMA stores.
"""

import numpy as np

DIM = 4
H = 256
R = 64
NF = 64
NCORES = 8
JS = 8                      # j points per core
N1, N2S, N3, N4 = 64, 8, 64, 32
NB = 8                      # branch batch
BI = NB * 64                # 512 output rows (b,i)
JKL = JS * 64 * 32          # 16384 output cols per core
MAGIC = 12582912.0          # 1.5 * 2**23 : fp32 round-to-nearest-int trick
TWO_PI = 6.283185307179586
INV_2PI = 1.0 / TWO_PI
HALF_PI = 1.5707963267948966

_CACHE = {}


def _build_nc(sim_compat=False):
    import concourse.mybir as mybir
    import concourse.tile as tile
    from concourse import bacc

    f32 = mybir.dt.float32
    f32r = mybir.dt.float32r  # noqa: F841  (kept for reference)
    bf16 = mybir.dt.bfloat16
    Alu = mybir.AluOpType
    Act = mybir.ActivationFunctionType

    nc = bacc.Bacc("TRN2", target_bir_lowering=False, debug=False,
                   num_devices=NCORES)

    def din(name, shape):
        return nc.dram_tensor(name, list(shape), f32, kind="ExternalInput").ap()

    x_in = {1: din("x1", (1, N1)), 2: din("x2s", (1, N2S)),
            3: din("x3", (1, N3)), 4: din("x4", (1, N4))}
    smallpk = din("smallpk", (128, 40))
    tw0T = din("tw0T", (128, DIM, 1, H))
    tw12T = din("tw12T", (128, DIM, 4, H))
    tw3T = din("tw3T", (128, DIM, 2, R))
    fT = din("fT", (128, 2, NB))
    bw012T = din("bw012T", (128, 3, 2, H))
    bw3T = din("bw3T", (128, 2, R))

    out = nc.dram_tensor("out", [BI, JKL], f32, kind="ExternalOutput").ap()

    with tile.TileContext(nc) as tc, \
         tc.tile_pool(name="consts", bufs=1) as consts, \
         tc.tile_pool(name="mlp", bufs=2) as mlp, \
         tc.tile_pool(name="fact", bufs=1) as fact, \
         tc.tile_pool(name="stag", bufs=3) as stag, \
         tc.tile_pool(name="pmlp", bufs=2, space="PSUM") as pmlp, \
         tc.tile_pool(name="pmain", bufs=6, space="PSUM") as pmain:

        def load(ap_dram, shape, name):
            t = consts.tile(list(shape), f32, tag=name, name=name)
            nc.sync.dma_start(t[:], ap_dram)
            return t

        sp = load(smallpk, (128, 40), "sp")
        tw0_sb = load(tw0T, (128, DIM, 1, H), "tw0")
        tw12_sb = load(tw12T, (128, DIM, 4, H), "tw12")
        tw3_sb = load(tw3T, (128, DIM, 2, R), "tw3")
        fT_sb = load(fT, (128, 2, NB), "fT")
        bw012_sb = load(bw012T, (128, 3, 2, H), "bw012")
        bw3_sb = load(bw3T, (128, 2, R), "bw3")

        # smallpk column map
        b2 = sp[:, 0:1]
        ph = sp[:, 1:2]
        tb = {(0, i): sp[:, 2 + 2 * i: 4 + 2 * i] for i in range(DIM)}
        tb.update({(1, i): sp[:, 10 + 2 * i: 12 + 2 * i] for i in range(DIM)})
        tb.update({(2, i): sp[:, 18 + 2 * i: 20 + 2 * i] for i in range(DIM)})
        bb = {k: sp[:, 26 + 2 * k: 28 + 2 * k] for k in range(3)}
        tb3 = {i: sp[0:R, 32 + i: 33 + i] for i in range(DIM)}
        bb3 = sp[0:R, 36:37]

        UT = fact.tile([128, BI], bf16, tag="UT", name="UT")
        nc.vector.memset(UT[:], 0.0)
        Wp = []
        for p in range(4):
            w = fact.tile([128, 2, 64, 32], bf16, tag=f"W{p}", name=f"W{p}")
            nc.gpsimd.memset(w[64:128], 0.0)
            Wp.append(w)

        def act_silu(out_chunk, ps, bias_col, n):
            if not sim_compat:
                nc.scalar.activation(out_chunk, ps, Act.Silu, bias=bias_col)
                return
            # CoreSim has no Silu: y*sigmoid(y), y = ps + bias
            y = mlp.tile([128, 64], f32, tag="sly", name="sly")[:ps.shape[0], :n]
            nc.vector.tensor_scalar(y, ps, bias_col, None, Alu.add)
            s = mlp.tile([128, 64], f32, tag="sls", name="sls")[:ps.shape[0], :n]
            nc.scalar.activation(s, y, Act.Sigmoid)
            nc.vector.tensor_mul(out_chunk, y, s)

        def dense(rhs_chunks, n, wsb, M, bias_ap2, out_chunks, act):
            # out = act(W @ rhs + b); wsb[:, ko, :] are the lhsT K-chunks
            KC = len(rhs_chunks)
            for mo in range((M + 127) // 128):
                mw = min(128, M - 128 * mo)
                ps = pmlp.tile([128, 64], f32, tag="pmlp",
                               name="pmlp")[:mw, :n]
                for ko in range(KC):
                    nc.tensor.matmul(ps,
                                     lhsT=wsb[:, ko, 128 * mo:128 * mo + mw],
                                     rhs=rhs_chunks[ko],
                                     start=(ko == 0), stop=(ko == KC - 1))
                bias_col = bias_ap2[:mw, mo:mo + 1]
                if act == "silu":
                    act_silu(out_chunks[mo], ps, bias_col, n)
                else:
                    nc.scalar.activation(out_chunks[mo], ps,
                                         Act.Identity, bias=bias_col)

        def fourier(x_ap, n):
            # rows 0:64 cos(xB), rows 64:128 sin(xB) via range-reduced Sin
            xr = mlp.tile([128, 64], f32, tag="xr", name="xr")[:, :n]
            nc.sync.dma_start(xr, x_ap.to_broadcast((128, n)))
            q = mlp.tile([128, 64], f32, tag="q", name="q")[:, :n]
            nc.vector.tensor_scalar(q, xr, b2, ph, Alu.mult, Alu.add)
            u = mlp.tile([128, 64], f32, tag="u", name="u")[:, :n]
            nc.vector.tensor_scalar(u, q, INV_2PI, MAGIC, Alu.mult, Alu.add)
            nc.vector.tensor_scalar(u, u, MAGIC, TWO_PI, Alu.subtract,
                                    Alu.mult)
            nc.vector.tensor_sub(q, q, u)
            ft = mlp.tile([128, 64], f32, tag="feat", name="feat")[:, :n]
            nc.scalar.activation(ft, q, Act.Sin)
            return ft

        def trunk(i, n):
            ft = fourier(x_in[i + 1], n)
            h1 = mlp.tile([128, 2, 64], f32, tag="h1", name="h1")[:, :, :n]
            dense([ft], n, tw0_sb[:, i], H, tb[(0, i)],
                  [h1[:, 0], h1[:, 1]], "silu")
            h2 = mlp.tile([128, 2, 64], f32, tag="h2", name="h2")[:, :, :n]
            dense([h1[:, 0], h1[:, 1]], n, tw12_sb[:, i, 0:2], H, tb[(1, i)],
                  [h2[:, 0], h2[:, 1]], "silu")
            h3 = mlp.tile([128, 2, 64], f32, tag="h3", name="h3")[:, :, :n]
            dense([h2[:, 0], h2[:, 1]], n, tw12_sb[:, i, 2:4], H, tb[(2, i)],
                  [h3[:, 0], h3[:, 1]], "silu")
            t = fact.tile([R, n], f32, tag=f"t{i}", name=f"t{i}")
            tb3_2 = tb3[i][:, :]  # [64,1] -> bias_ap2 with one column
            dense([h3[:, 0], h3[:, 1]], n, tw3_sb[:, i], R, tb3_2, [t[:]],
                  "linear")
            return t

        # W-path trunks first so the Khatri-Rao operand is ready earliest
        t2T = trunk(1, N2S)
        t3T = trunk(2, N3)
        G = fact.tile([R, JS * 64], f32, tag="G", name="G")
        for jl in range(JS):
            nc.vector.tensor_scalar(G[:, 64 * jl:64 * (jl + 1)], t3T[:],
                                    t2T[:, jl:jl + 1], None, Alu.mult)
        t4T = trunk(3, N4)

        # branch MLP (no Fourier features)
        h1 = mlp.tile([128, 2, NB], f32, tag="bh1", name="bh1")
        dense([fT_sb[:, 0], fT_sb[:, 1]], NB, bw012_sb[:, 0], H, bb[0],
              [h1[:, 0], h1[:, 1]], "silu")
        h2 = mlp.tile([128, 2, NB], f32, tag="bh2", name="bh2")
        dense([h1[:, 0], h1[:, 1]], NB, bw012_sb[:, 1], H, bb[1],
              [h2[:, 0], h2[:, 1]], "silu")
        h3 = mlp.tile([128, 2, NB], f32, tag="bh3", name="bh3")
        dense([h2[:, 0], h2[:, 1]], NB, bw012_sb[:, 2], H, bb[2],
              [h3[:, 0], h3[:, 1]], "silu")
        bT = fact.tile([R, NB], f32, tag="bT", name="bT")
        dense([h3[:, 0], h3[:, 1]], NB, bw3_sb, R, bb3[:, :], [bT[:]],
              "linear")

        t1T = trunk(0, N1)
        for b in range(NB):
            nc.vector.tensor_scalar(UT[0:R, 64 * b:64 * (b + 1)], t1T[:],
                                    bT[:, b:b + 1], None, Alu.mult)

        # main contraction, one jl-pair at a time so W gen overlaps matmuls
        Gv = G[:].rearrange("p (a b) -> p a b", a=JS)
        for p in range(4):
            w = Wp[p]
            for s in range(2):
                jl = 2 * p + s
                nc.vector.tensor_tensor(
                    w[0:R, s],
                    Gv[:, jl, :, None].to_broadcast((R, 64, 32)),
                    t4T[:, None, :].to_broadcast((R, 64, 32)),
                    Alu.mult)
            wf = w[:].rearrange("p a b c -> p (a b c)")
            for m in range(BI // 128):
                lhsT = UT[:, 128 * m:128 * (m + 1)]
                stg = stag.tile([128, 4096], f32, tag="stg", name="stg")
                for qi in range(8):
                    ps = pmain.tile([128, 512], f32, tag="pmain", name="pmain")
                    nc.tensor.matmul(
                        ps[:], lhsT=lhsT,
                        rhs=wf[:, 512 * qi:512 * (qi + 1)],
                        start=True, stop=True)
                    nc.any.tensor_copy(stg[:, 512 * qi:512 * (qi + 1)], ps[:])
                nc.sync.dma_start(
                    out[128 * m:128 * (m + 1), 4096 * p:4096 * (p + 1)],
                    stg[:])

    nc.compile()
    return nc


def _prep_maps(inputs):
    def f32c(a):
        return np.ascontiguousarray(np.asarray(a), dtype=np.float32)

    i = {k: np.asarray(v) for k, v in inputs.items()}
    B = f32c(i["B"]).reshape(NF)

    sp = np.zeros((128, 40), np.float32)
    sp[:NF, 0] = B
    sp[NF:, 0] = B
    sp[:NF, 1] = HALF_PI        # cos rows get sin(x + pi/2)
    for k, key in enumerate(("tb0", "tb1", "tb2")):
        for t in range(DIM):
            sp[:, 2 + 8 * k + 2 * t: 4 + 8 * k + 2 * t] = \
                f32c(i[key][t]).reshape(2, 128).T
    for k, key in enumerate(("bb0", "bb1", "bb2")):
        sp[:, 26 + 2 * k: 28 + 2 * k] = f32c(i[key]).reshape(2, 128).T
    for t in range(DIM):
        sp[:R, 32 + t] = f32c(i["tb3"][t]).reshape(R)
    sp[:R, 36] = f32c(i["bb3"]).reshape(R)

    def wT(w, kc, m):            # (m, kc*128) -> (128, kc, m)
        return f32c(w).T.reshape(kc, 128, m).transpose(1, 0, 2)

    common = {
        "x1": f32c(i["x1"]).reshape(1, N1),
        "x3": f32c(i["x3"]).reshape(1, N3),
        "x4": f32c(i["x4"]).reshape(1, N4),
        "smallpk": sp,
        # (128, DIM, KC, M) stacks
        "tw0T": np.stack([wT(i["tW0"][t], 1, H) for t in range(DIM)], axis=1),
        "tw12T": np.stack(
            [np.concatenate([wT(i["tW1"][t], 2, H), wT(i["tW2"][t], 2, H)],
                            axis=1) for t in range(DIM)], axis=1),
        "tw3T": np.stack([wT(i["tW3"][t], 2, R) for t in range(DIM)], axis=1),
        "fT": f32c(i["f"]).T.reshape(2, 128, NB).transpose(1, 0, 2),
        "bw012T": np.stack([wT(i["bW0"], 2, H), wT(i["bW1"], 2, H),
                            wT(i["bW2"], 2, H)], axis=1),
        "bw3T": wT(i["bW3"], 2, R),
    }
    common = {k: np.ascontiguousarray(v, dtype=np.float32)
              for k, v in common.items()}
    x2 = f32c(i["x2"]).reshape(64)
    maps = []
    for c in range(NCORES):
        m = dict(common)
        m["x2s"] = np.ascontiguousarray(
            x2[JS * c:JS * (c + 1)].reshape(1, JS))
        maps.append(m)
    return maps


def _gather(results):
    full = np.empty((NB, 64, 64, 64, 32, 1), np.float32)
    for c in range(NCORES):
        full[:, :, JS * c:JS * (c + 1), :, :, 0] = \
            results[c]["out"].reshape(NB, 64, JS, 64, 32)
    return full


def kernel(**inputs):
    from concourse.bass_utils import run_bass_kernel_spmd

    if "nc" not in _CACHE:
        _CACHE["nc"] = _build_nc()
    nc = _CACHE["nc"]
    maps = _prep_maps(inputs)
    res = run_bass_kernel_spmd(nc, maps, core_ids=list(range(NCORES)))
    return _gather(res.results)


# revision 7
# speedup vs baseline: 1.0571x; 1.0571x over previous
"""DeepOHeat-ST rank-contraction kernel for 8x Trainium2 NeuronCores.

Computes einsum('iyz,jyz,kyz,lyz,byz->bijkly', t1,t2,t3,t4,b) where the
t_i / b factors come from tiny Fourier-feature MLP trunks and a branch MLP.

Sharding: data-parallel over the j axis (trunk-2 evaluation points).  Each
of the 8 cores receives its 8 rows of x2, evaluates all MLPs (replicated,
~1% of the FLOPs), builds
    U^T[z, b*64+i] = bT[z,b] * t1T[z,i]                    (512 cols)
    W_p[z, (jl*64+k)*32+l] = t2T[z,jl]*t3T[z,k]*t4T[z,l]   (4x 4096 cols)
and evaluates its 512x16384 output shard as K=64 (zero-padded to 128)
bf16 matmuls (N=512), PSUM -> SBUF staging -> 2MB DMA stores.
"""

import numpy as np

DIM = 4
H = 256
R = 64
NF = 64
NCORES = 8
JS = 8                      # j points per core
N1, N2S, N3, N4 = 64, 8, 64, 32
NB = 8                      # branch batch
BI = NB * 64                # 512 output rows (b,i)
JKL = JS * 64 * 32          # 16384 output cols per core
MAGIC = 12582912.0          # 1.5 * 2**23 : fp32 round-to-nearest-int trick
TWO_PI = 6.283185307179586
INV_2PI = 1.0 / TWO_PI
HALF_PI = 1.5707963267948966

_CACHE = {}


def _build_nc(sim_compat=False):
    import concourse.mybir as mybir
    import concourse.tile as tile
    from concourse import bacc

    f32 = mybir.dt.float32
    f32r = mybir.dt.float32r  # noqa: F841  (kept for reference)
    bf16 = mybir.dt.float16  # fp16: same PE rate as bf16, 8x finer mantissa
    Alu = mybir.AluOpType
    Act = mybir.ActivationFunctionType

    nc = bacc.Bacc("TRN2", target_bir_lowering=False, debug=False,
                   num_devices=NCORES)

    def din(name, shape, dt=None):
        return nc.dram_tensor(name, list(shape), dt or f32,
                              kind="ExternalInput").ap()

    x_in = {1: din("x1", (1, N1)), 2: din("x2s", (1, N2S)),
            3: din("x3", (1, N3)), 4: din("x4", (1, N4))}
    smallpk = din("smallpk", (128, 40))
    tw0T = din("tw0T", (128, DIM, 1, H), bf16)
    tw12T = din("tw12T", (128, DIM, 4, H), bf16)
    tw3T = din("tw3T", (128, DIM, 2, R), bf16)
    fT = din("fT", (128, 2, NB), bf16)
    bw012T = din("bw012T", (128, 3, 2, H), bf16)
    bw3T = din("bw3T", (128, 2, R), bf16)

    out = nc.dram_tensor("out", [BI, JKL], f32, kind="ExternalOutput").ap()

    with tile.TileContext(nc) as tc, \
         tc.tile_pool(name="consts", bufs=1) as consts, \
         tc.tile_pool(name="mlp", bufs=2) as mlp, \
         tc.tile_pool(name="fact", bufs=1) as fact, \
         tc.tile_pool(name="stag", bufs=3) as stag, \
         tc.tile_pool(name="pmlp", bufs=2, space="PSUM") as pmlp, \
         tc.tile_pool(name="pmain", bufs=6, space="PSUM") as pmain:

        def load(ap_dram, shape, name, dt=None):
            t = consts.tile(list(shape), dt or f32, tag=name, name=name)
            nc.sync.dma_start(t[:], ap_dram)
            return t

        sp = load(smallpk, (128, 40), "sp")
        tw0_sb = load(tw0T, (128, DIM, 1, H), "tw0", bf16)
        tw12_sb = load(tw12T, (128, DIM, 4, H), "tw12", bf16)
        tw3_sb = load(tw3T, (128, DIM, 2, R), "tw3", bf16)
        fT_sb = load(fT, (128, 2, NB), "fT", bf16)
        bw012_sb = load(bw012T, (128, 3, 2, H), "bw012", bf16)
        bw3_sb = load(bw3T, (128, 2, R), "bw3", bf16)

        # smallpk column map
        b2 = sp[:, 0:1]
        ph = sp[:, 1:2]
        tb = {(0, i): sp[:, 2 + 2 * i: 4 + 2 * i] for i in range(DIM)}
        tb.update({(1, i): sp[:, 10 + 2 * i: 12 + 2 * i] for i in range(DIM)})
        tb.update({(2, i): sp[:, 18 + 2 * i: 20 + 2 * i] for i in range(DIM)})
        bb = {k: sp[:, 26 + 2 * k: 28 + 2 * k] for k in range(3)}
        tb3 = {i: sp[0:R, 32 + i: 33 + i] for i in range(DIM)}
        bb3 = sp[0:R, 36:37]

        UT = fact.tile([128, BI], bf16, tag="UT", name="UT")
        nc.vector.memset(UT[:], 0.0)
        Wp = []
        for p in range(4):
            w = fact.tile([128, 2, 64, 32], bf16, tag=f"W{p}", name=f"W{p}")
            nc.gpsimd.memset(w[64:128], 0.0)
            Wp.append(w)

        def act_silu(out_chunk, ps, bias_col, n):
            if not sim_compat:
                nc.scalar.activation(out_chunk, ps, Act.Silu, bias=bias_col)
                return
            # CoreSim has no Silu: y*sigmoid(y), y = ps + bias
            y = mlp.tile([128, 64], f32, tag="sly", name="sly")[:ps.shape[0], :n]
            nc.vector.tensor_scalar(y, ps, bias_col, None, Alu.add)
            s = mlp.tile([128, 64], f32, tag="sls", name="sls")[:ps.shape[0], :n]
            nc.scalar.activation(s, y, Act.Sigmoid)
            nc.vector.tensor_mul(out_chunk, y, s)

        def dense(rhs_chunks, n, wsb, M, bias_ap2, out_chunks, act):
            # out = act(W @ rhs + b); wsb[:, ko, :] are the lhsT K-chunks
            KC = len(rhs_chunks)
            for mo in range((M + 127) // 128):
                mw = min(128, M - 128 * mo)
                ps = pmlp.tile([128, 64], f32, tag="pmlp",
                               name="pmlp")[:mw, :n]
                for ko in range(KC):
                    nc.tensor.matmul(ps,
                                     lhsT=wsb[:, ko, 128 * mo:128 * mo + mw],
                                     rhs=rhs_chunks[ko],
                                     start=(ko == 0), stop=(ko == KC - 1))
                bias_col = bias_ap2[:mw, mo:mo + 1]
                if act == "silu":
                    act_silu(out_chunks[mo], ps, bias_col, n)
                else:
                    nc.scalar.activation(out_chunks[mo], ps,
                                         Act.Identity, bias=bias_col)

        def fourier(x_ap, n):
            # rows 0:64 cos(xB), rows 64:128 sin(xB) via range-reduced Sin
            xr = mlp.tile([128, 64], f32, tag="xr", name="xr")[:, :n]
            nc.sync.dma_start(xr, x_ap.to_broadcast((128, n)))
            q = mlp.tile([128, 64], f32, tag="q", name="q")[:, :n]
            nc.vector.tensor_scalar(q, xr, b2, ph, Alu.mult, Alu.add)
            u = mlp.tile([128, 64], f32, tag="u", name="u")[:, :n]
            nc.vector.tensor_scalar(u, q, INV_2PI, MAGIC, Alu.mult, Alu.add)
            nc.vector.tensor_scalar(u, u, MAGIC, TWO_PI, Alu.subtract,
                                    Alu.mult)
            nc.vector.tensor_sub(q, q, u)
            ft = mlp.tile([128, 64], bf16, tag="feat", name="feat")[:, :n]
            nc.scalar.activation(ft, q, Act.Sin)
            return ft

        def trunk(i, n):
            ft = fourier(x_in[i + 1], n)
            h1 = mlp.tile([128, 2, 64], bf16, tag="h1", name="h1")[:, :, :n]
            dense([ft], n, tw0_sb[:, i], H, tb[(0, i)],
                  [h1[:, 0], h1[:, 1]], "silu")
            h2 = mlp.tile([128, 2, 64], bf16, tag="h2", name="h2")[:, :, :n]
            dense([h1[:, 0], h1[:, 1]], n, tw12_sb[:, i, 0:2], H, tb[(1, i)],
                  [h2[:, 0], h2[:, 1]], "silu")
            h3 = mlp.tile([128, 2, 64], bf16, tag="h3", name="h3")[:, :, :n]
            dense([h2[:, 0], h2[:, 1]], n, tw12_sb[:, i, 2:4], H, tb[(2, i)],
                  [h3[:, 0], h3[:, 1]], "silu")
            t = fact.tile([R, n], f32, tag=f"t{i}", name=f"t{i}")
            tb3_2 = tb3[i][:, :]  # [64,1] -> bias_ap2 with one column
            dense([h3[:, 0], h3[:, 1]], n, tw3_sb[:, i], R, tb3_2, [t[:]],
                  "linear")
            return t

        # W-path trunks first so the Khatri-Rao operand is ready earliest
        t2T = trunk(1, N2S)
        t3T = trunk(2, N3)
        G = fact.tile([R, JS * 64], f32, tag="G", name="G")
        for jl in range(JS):
            nc.vector.tensor_scalar(G[:, 64 * jl:64 * (jl + 1)], t3T[:],
                                    t2T[:, jl:jl + 1], None, Alu.mult)
        t4T = trunk(3, N4)

        # branch MLP (no Fourier features)
        h1 = mlp.tile([128, 2, NB], bf16, tag="bh1", name="bh1")
        dense([fT_sb[:, 0], fT_sb[:, 1]], NB, bw012_sb[:, 0], H, bb[0],
              [h1[:, 0], h1[:, 1]], "silu")
        h2 = mlp.tile([128, 2, NB], bf16, tag="bh2", name="bh2")
        dense([h1[:, 0], h1[:, 1]], NB, bw012_sb[:, 1], H, bb[1],
              [h2[:, 0], h2[:, 1]], "silu")
        h3 = mlp.tile([128, 2, NB], bf16, tag="bh3", name="bh3")
        dense([h2[:, 0], h2[:, 1]], NB, bw012_sb[:, 2], H, bb[2],
              [h3[:, 0], h3[:, 1]], "silu")
        bT = fact.tile([R, NB], f32, tag="bT", name="bT")
        dense([h3[:, 0], h3[:, 1]], NB, bw3_sb, R, bb3[:, :], [bT[:]],
              "linear")

        t1T = trunk(0, N1)
        for b in range(NB):
            nc.vector.tensor_scalar(UT[0:R, 64 * b:64 * (b + 1)], t1T[:],
                                    bT[:, b:b + 1], None, Alu.mult)

        # main contraction, one jl-pair at a time so W gen overlaps matmuls
        Gv = G[:].rearrange("p (a b) -> p a b", a=JS)
        for p in range(4):
            w = Wp[p]
            for s in range(2):
                jl = 2 * p + s
                nc.vector.tensor_tensor(
                    w[0:R, s],
                    Gv[:, jl, :, None].to_broadcast((R, 64, 32)),
                    t4T[:, None, :].to_broadcast((R, 64, 32)),
                    Alu.mult)
            wf = w[:].rearrange("p a b c -> p (a b c)")
            for m in range(BI // 128):
                lhsT = UT[:, 128 * m:128 * (m + 1)]
                stg = stag.tile([128, 4096], f32, tag="stg", name="stg")
                for qi in range(8):
                    ps = pmain.tile([128, 512], f32, tag="pmain", name="pmain")
                    nc.tensor.matmul(
                        ps[:], lhsT=lhsT,
                        rhs=wf[:, 512 * qi:512 * (qi + 1)],
                        start=True, stop=True)
                    nc.any.tensor_copy(stg[:, 512 * qi:512 * (qi + 1)], ps[:])
                nc.sync.dma_start(
                    out[128 * m:128 * (m + 1), 4096 * p:4096 * (p + 1)],
                    stg[:])

    nc.compile()
    return nc


def _prep_maps(inputs):
    def f32c(a):
        return np.ascontiguousarray(np.asarray(a), dtype=np.float32)

    i = {k: np.asarray(v) for k, v in inputs.items()}
    B = f32c(i["B"]).reshape(NF)

    sp = np.zeros((128, 40), np.float32)
    sp[:NF, 0] = B
    sp[NF:, 0] = B
    sp[:NF, 1] = HALF_PI        # cos rows get sin(x + pi/2)
    for k, key in enumerate(("tb0", "tb1", "tb2")):
        for t in range(DIM):
            sp[:, 2 + 8 * k + 2 * t: 4 + 8 * k + 2 * t] = \
                f32c(i[key][t]).reshape(2, 128).T
    for k, key in enumerate(("bb0", "bb1", "bb2")):
        sp[:, 26 + 2 * k: 28 + 2 * k] = f32c(i[key]).reshape(2, 128).T
    for t in range(DIM):
        sp[:R, 32 + t] = f32c(i["tb3"][t]).reshape(R)
    sp[:R, 36] = f32c(i["bb3"]).reshape(R)

    def wT(w, kc, m):            # (m, kc*128) -> (128, kc, m)
        return f32c(w).T.reshape(kc, 128, m).transpose(1, 0, 2)

    common = {
        "x1": f32c(i["x1"]).reshape(1, N1),
        "x3": f32c(i["x3"]).reshape(1, N3),
        "x4": f32c(i["x4"]).reshape(1, N4),
        "smallpk": sp,
        # (128, DIM, KC, M) stacks
        "tw0T": np.stack([wT(i["tW0"][t], 1, H) for t in range(DIM)], axis=1),
        "tw12T": np.stack(
            [np.concatenate([wT(i["tW1"][t], 2, H), wT(i["tW2"][t], 2, H)],
                            axis=1) for t in range(DIM)], axis=1),
        "tw3T": np.stack([wT(i["tW3"][t], 2, R) for t in range(DIM)], axis=1),
        "fT": f32c(i["f"]).T.reshape(2, 128, NB).transpose(1, 0, 2),
        "bw012T": np.stack([wT(i["bW0"], 2, H), wT(i["bW1"], 2, H),
                            wT(i["bW2"], 2, H)], axis=1),
        "bw3T": wT(i["bW3"], 2, R),
    }
    bf = ("tw0T", "tw12T", "tw3T", "fT", "bw012T", "bw3T")
    common = {k: np.ascontiguousarray(
                  v, dtype=(np.float16 if k in bf else np.float32))
              for k, v in common.items()}
    x2 = f32c(i["x2"]).reshape(64)
    maps = []
    for c in range(NCORES):
        m = dict(common)
        m["x2s"] = np.ascontiguousarray(
            x2[JS * c:JS * (c + 1)].reshape(1, JS))
        maps.append(m)
    return maps


def _gather(results):
    full = np.empty((NB, 64, 64, 64, 32, 1), np.float32)
    for c in range(NCORES):
        full[:, :, JS * c:JS * (c + 1), :, :, 0] = \
            results[c]["out"].reshape(NB, 64, JS, 64, 32)
    return full


def kernel(**inputs):
    from concourse.bass_utils import run_bass_kernel_spmd

    if "nc" not in _CACHE:
        _CACHE["nc"] = _build_nc()
    nc = _CACHE["nc"]
    maps = _prep_maps(inputs)
    res = run_bass_kernel_spmd(nc, maps, core_ids=list(range(NCORES)))
    return _gather(res.results)


# revision 8
# speedup vs baseline: 1.1402x; 1.0786x over previous
"""DeepOHeat-ST rank-contraction kernel for 8x Trainium2 NeuronCores.

Computes einsum('iyz,jyz,kyz,lyz,byz->bijkly', t1,t2,t3,t4,b) where the
t_i / b factors come from tiny Fourier-feature MLP trunks and a branch MLP.

Sharding: data-parallel over the j axis (trunk-2 evaluation points).  Each
of the 8 cores receives its 8 rows of x2, evaluates all MLPs (replicated,
~1% of the FLOPs), builds
    U^T[z, b*64+i] = bT[z,b] * t1T[z,i]                    (512 cols)
    W_p[z, (jl*64+k)*32+l] = t2T[z,jl]*t3T[z,k]*t4T[z,l]   (4x 4096 cols)
and evaluates its 512x16384 output shard as K=64 (zero-padded to 128)
bf16 matmuls (N=512), PSUM -> SBUF staging -> 2MB DMA stores.
"""

import numpy as np

DIM = 4
H = 256
R = 64
NF = 64
NCORES = 8
JS = 8                      # j points per core
N1, N2S, N3, N4 = 64, 8, 64, 32
NB = 8                      # branch batch
BI = NB * 64                # 512 output rows (b,i)
JKL = JS * 64 * 32          # 16384 output cols per core
MAGIC = 12582912.0          # 1.5 * 2**23 : fp32 round-to-nearest-int trick
TWO_PI = 6.283185307179586
INV_2PI = 1.0 / TWO_PI
HALF_PI = 1.5707963267948966

_CACHE = {}


def _build_nc(sim_compat=False):
    import concourse.mybir as mybir
    import concourse.tile as tile
    from concourse import bacc

    f32 = mybir.dt.float32
    f32r = mybir.dt.float32r  # noqa: F841  (kept for reference)
    bf16 = mybir.dt.float16  # fp16: same PE rate as bf16, 8x finer mantissa
    Alu = mybir.AluOpType
    Act = mybir.ActivationFunctionType

    nc = bacc.Bacc("TRN2", target_bir_lowering=False, debug=False,
                   num_devices=NCORES)

    def din(name, shape, dt=None):
        return nc.dram_tensor(name, list(shape), dt or f32,
                              kind="ExternalInput").ap()

    x_in = {1: din("x1", (1, N1)), 2: din("x2s", (1, N2S)),
            3: din("x3", (1, N3)), 4: din("x4", (1, N4))}
    smallpk = din("smallpk", (128, 40))
    tw0T = din("tw0T", (128, DIM, 1, H), bf16)
    tw12T = din("tw12T", (128, DIM, 4, H), bf16)
    tw3T = din("tw3T", (128, DIM, 2, R), bf16)
    fT = din("fT", (128, 2, NB), bf16)
    bw012T = din("bw012T", (128, 3, 2, H), bf16)
    bw3T = din("bw3T", (128, 2, R), bf16)

    out = nc.dram_tensor("out", [BI, JKL], f32, kind="ExternalOutput").ap()

    with tile.TileContext(nc) as tc, \
         tc.tile_pool(name="consts", bufs=1) as consts, \
         tc.tile_pool(name="mlp", bufs=3) as mlp, \
         tc.tile_pool(name="fact", bufs=1) as fact, \
         tc.tile_pool(name="stag", bufs=4) as stag, \
         tc.tile_pool(name="psum", bufs=4, space="PSUM") as psum:

        def load(ap_dram, shape, name, dt=None):
            t = consts.tile(list(shape), dt or f32, tag=name, name=name)
            nc.sync.dma_start(t[:], ap_dram)
            return t

        sp = load(smallpk, (128, 40), "sp")
        tw0_sb = load(tw0T, (128, DIM, 1, H), "tw0", bf16)
        tw12_sb = load(tw12T, (128, DIM, 4, H), "tw12", bf16)
        tw3_sb = load(tw3T, (128, DIM, 2, R), "tw3", bf16)
        fT_sb = load(fT, (128, 2, NB), "fT", bf16)
        bw012_sb = load(bw012T, (128, 3, 2, H), "bw012", bf16)
        bw3_sb = load(bw3T, (128, 2, R), "bw3", bf16)

        # smallpk column map
        b2 = sp[:, 0:1]
        ph = sp[:, 1:2]
        tb = {(0, i): sp[:, 2 + 2 * i: 4 + 2 * i] for i in range(DIM)}
        tb.update({(1, i): sp[:, 10 + 2 * i: 12 + 2 * i] for i in range(DIM)})
        tb.update({(2, i): sp[:, 18 + 2 * i: 20 + 2 * i] for i in range(DIM)})
        bb = {k: sp[:, 26 + 2 * k: 28 + 2 * k] for k in range(3)}
        tb3 = {i: sp[0:R, 32 + i: 33 + i] for i in range(DIM)}
        bb3 = sp[0:R, 36:37]

        UT = fact.tile([128, BI], bf16, tag="UT", name="UT")
        nc.vector.memset(UT[:], 0.0)
        Wp = []
        for p in range(4):
            w = fact.tile([128, 2, 64, 32], bf16, tag=f"W{p}", name=f"W{p}")
            nc.gpsimd.memset(w[64:128], 0.0)
            Wp.append(w)

        def act_silu(out_chunk, ps, bias_col, n):
            if not sim_compat:
                nc.scalar.activation(out_chunk, ps, Act.Silu, bias=bias_col)
                return
            # CoreSim has no Silu: y*sigmoid(y), y = ps + bias
            y = mlp.tile([128, 64], f32, tag="sly", name="sly")[:ps.shape[0], :n]
            nc.vector.tensor_scalar(y, ps, bias_col, None, Alu.add)
            s = mlp.tile([128, 64], f32, tag="sls", name="sls")[:ps.shape[0], :n]
            nc.scalar.activation(s, y, Act.Sigmoid)
            nc.vector.tensor_mul(out_chunk, y, s)

        def dense(rhs_chunks, n, wsb, M, bias_ap2, out_chunks, act):
            # out = act(W @ rhs + b); wsb[:, ko, :] are the lhsT K-chunks
            KC = len(rhs_chunks)
            for mo in range((M + 127) // 128):
                mw = min(128, M - 128 * mo)
                ps = psum.tile([128, 1024], f32, tag="ps",
                               name="ps")[:mw, :n]
                for ko in range(KC):
                    nc.tensor.matmul(ps,
                                     lhsT=wsb[:, ko, 128 * mo:128 * mo + mw],
                                     rhs=rhs_chunks[ko],
                                     start=(ko == 0), stop=(ko == KC - 1))
                bias_col = bias_ap2[:mw, mo:mo + 1]
                if act == "silu":
                    act_silu(out_chunks[mo], ps, bias_col, n)
                else:
                    nc.scalar.activation(out_chunks[mo], ps,
                                         Act.Identity, bias=bias_col)

        def fourier(x_ap, n):
            # rows 0:64 cos(xB), rows 64:128 sin(xB) via range-reduced Sin
            xr = mlp.tile([128, 64], f32, tag="xr", name="xr")[:, :n]
            nc.sync.dma_start(xr, x_ap.to_broadcast((128, n)))
            q = mlp.tile([128, 64], f32, tag="q", name="q")[:, :n]
            nc.vector.tensor_scalar(q, xr, b2, ph, Alu.mult, Alu.add)
            u = mlp.tile([128, 64], f32, tag="u", name="u")[:, :n]
            nc.vector.tensor_scalar(u, q, INV_2PI, MAGIC, Alu.mult, Alu.add)
            nc.vector.tensor_scalar(u, u, MAGIC, TWO_PI, Alu.subtract,
                                    Alu.mult)
            nc.vector.tensor_sub(q, q, u)
            ft = mlp.tile([128, 64], bf16, tag="feat", name="feat")[:, :n]
            nc.scalar.activation(ft, q, Act.Sin)
            return ft

        def trunk(i, n):
            ft = fourier(x_in[i + 1], n)
            h1 = mlp.tile([128, 2, 64], bf16, tag="h1", name="h1")[:, :, :n]
            dense([ft], n, tw0_sb[:, i], H, tb[(0, i)],
                  [h1[:, 0], h1[:, 1]], "silu")
            h2 = mlp.tile([128, 2, 64], bf16, tag="h2", name="h2")[:, :, :n]
            dense([h1[:, 0], h1[:, 1]], n, tw12_sb[:, i, 0:2], H, tb[(1, i)],
                  [h2[:, 0], h2[:, 1]], "silu")
            h3 = mlp.tile([128, 2, 64], bf16, tag="h3", name="h3")[:, :, :n]
            dense([h2[:, 0], h2[:, 1]], n, tw12_sb[:, i, 2:4], H, tb[(2, i)],
                  [h3[:, 0], h3[:, 1]], "silu")
            t = fact.tile([R, n], f32, tag=f"t{i}", name=f"t{i}")
            tb3_2 = tb3[i][:, :]  # [64,1] -> bias_ap2 with one column
            dense([h3[:, 0], h3[:, 1]], n, tw3_sb[:, i], R, tb3_2, [t[:]],
                  "linear")
            return t

        # W-path trunks first so the Khatri-Rao operand is ready earliest
        t2T = trunk(1, N2S)
        t3T = trunk(2, N3)
        G = fact.tile([R, JS * 64], f32, tag="G", name="G")
        for jl in range(JS):
            nc.vector.tensor_scalar(G[:, 64 * jl:64 * (jl + 1)], t3T[:],
                                    t2T[:, jl:jl + 1], None, Alu.mult)
        t4T = trunk(3, N4)

        # branch MLP (no Fourier features)
        h1 = mlp.tile([128, 2, NB], bf16, tag="bh1", name="bh1")
        dense([fT_sb[:, 0], fT_sb[:, 1]], NB, bw012_sb[:, 0], H, bb[0],
              [h1[:, 0], h1[:, 1]], "silu")
        h2 = mlp.tile([128, 2, NB], bf16, tag="bh2", name="bh2")
        dense([h1[:, 0], h1[:, 1]], NB, bw012_sb[:, 1], H, bb[1],
              [h2[:, 0], h2[:, 1]], "silu")
        h3 = mlp.tile([128, 2, NB], bf16, tag="bh3", name="bh3")
        dense([h2[:, 0], h2[:, 1]], NB, bw012_sb[:, 2], H, bb[2],
              [h3[:, 0], h3[:, 1]], "silu")
        bT = fact.tile([R, NB], f32, tag="bT", name="bT")
        dense([h3[:, 0], h3[:, 1]], NB, bw3_sb, R, bb3[:, :], [bT[:]],
              "linear")

        t1T = trunk(0, N1)
        for b in range(NB):
            nc.vector.tensor_scalar(UT[0:R, 64 * b:64 * (b + 1)], t1T[:],
                                    bT[:, b:b + 1], None, Alu.mult)

        # main contraction, one jl-pair at a time so W gen overlaps matmuls
        Gv = G[:].rearrange("p (a b) -> p a b", a=JS)
        for p in range(4):
            w = Wp[p]
            for s in range(2):
                jl = 2 * p + s
                nc.vector.tensor_tensor(
                    w[0:R, s],
                    Gv[:, jl, :, None].to_broadcast((R, 64, 32)),
                    t4T[:, None, :].to_broadcast((R, 64, 32)),
                    Alu.mult)
            wf = w[:].rearrange("p a b c -> p (a b c)")
            for m in range(BI // 128):
                lhsT = UT[:, 128 * m:128 * (m + 1)]
                stg = stag.tile([128, 4096], f32, tag="stg", name="stg")
                for qi in range(4):
                    ps = psum.tile([128, 1024], f32, tag="ps", name="ps")
                    for h in range(2):
                        nc.tensor.matmul(
                            ps[:, 512 * h:512 * (h + 1)], lhsT=lhsT,
                            rhs=wf[:, 1024 * qi + 512 * h:
                                   1024 * qi + 512 * (h + 1)],
                            start=True, stop=True)
                    nc.any.tensor_copy(
                        stg[:, 1024 * qi:1024 * (qi + 1)], ps[:])
                nc.sync.dma_start(
                    out[128 * m:128 * (m + 1), 4096 * p:4096 * (p + 1)],
                    stg[:])

    nc.compile()
    return nc


def _prep_maps(inputs):
    def f32c(a):
        return np.ascontiguousarray(np.asarray(a), dtype=np.float32)

    i = {k: np.asarray(v) for k, v in inputs.items()}
    B = f32c(i["B"]).reshape(NF)

    sp = np.zeros((128, 40), np.float32)
    sp[:NF, 0] = B
    sp[NF:, 0] = B
    sp[:NF, 1] = HALF_PI        # cos rows get sin(x + pi/2)
    for k, key in enumerate(("tb0", "tb1", "tb2")):
        for t in range(DIM):
            sp[:, 2 + 8 * k + 2 * t: 4 + 8 * k + 2 * t] = \
                f32c(i[key][t]).reshape(2, 128).T
    for k, key in enumerate(("bb0", "bb1", "bb2")):
        sp[:, 26 + 2 * k: 28 + 2 * k] = f32c(i[key]).reshape(2, 128).T
    for t in range(DIM):
        sp[:R, 32 + t] = f32c(i["tb3"][t]).reshape(R)
    sp[:R, 36] = f32c(i["bb3"]).reshape(R)

    def wT(w, kc, m):            # (m, kc*128) -> (128, kc, m)
        return f32c(w).T.reshape(kc, 128, m).transpose(1, 0, 2)

    common = {
        "x1": f32c(i["x1"]).reshape(1, N1),
        "x3": f32c(i["x3"]).reshape(1, N3),
        "x4": f32c(i["x4"]).reshape(1, N4),
        "smallpk": sp,
        # (128, DIM, KC, M) stacks
        "tw0T": np.stack([wT(i["tW0"][t], 1, H) for t in range(DIM)], axis=1),
        "tw12T": np.stack(
            [np.concatenate([wT(i["tW1"][t], 2, H), wT(i["tW2"][t], 2, H)],
                            axis=1) for t in range(DIM)], axis=1),
        "tw3T": np.stack([wT(i["tW3"][t], 2, R) for t in range(DIM)], axis=1),
        "fT": f32c(i["f"]).T.reshape(2, 128, NB).transpose(1, 0, 2),
        "bw012T": np.stack([wT(i["bW0"], 2, H), wT(i["bW1"], 2, H),
                            wT(i["bW2"], 2, H)], axis=1),
        "bw3T": wT(i["bW3"], 2, R),
    }
    bf = ("tw0T", "tw12T", "tw3T", "fT", "bw012T", "bw3T")
    common = {k: np.ascontiguousarray(
                  v, dtype=(np.float16 if k in bf else np.float32))
              for k, v in common.items()}
    x2 = f32c(i["x2"]).reshape(64)
    maps = []
    for c in range(NCORES):
        m = dict(common)
        m["x2s"] = np.ascontiguousarray(
            x2[JS * c:JS * (c + 1)].reshape(1, JS))
        maps.append(m)
    return maps


def _gather(results):
    full = np.empty((NB, 64, 64, 64, 32, 1), np.float32)
    for c in range(NCORES):
        full[:, :, JS * c:JS * (c + 1), :, :, 0] = \
            results[c]["out"].reshape(NB, 64, JS, 64, 32)
    return full


def kernel(**inputs):
    from concourse.bass_utils import run_bass_kernel_spmd

    if "nc" not in _CACHE:
        _CACHE["nc"] = _build_nc()
    nc = _CACHE["nc"]
    maps = _prep_maps(inputs)
    res = run_bass_kernel_spmd(nc, maps, core_ids=list(range(NCORES)))
    return _gather(res.results)


# revision 9
# speedup vs baseline: 1.1464x; 1.0054x over previous
"""DeepOHeat-ST rank-contraction kernel for 8x Trainium2 NeuronCores.

Computes einsum('iyz,jyz,kyz,lyz,byz->bijkly', t1,t2,t3,t4,b) where the
t_i / b factors come from tiny Fourier-feature MLP trunks and a branch MLP.

Sharding: data-parallel over the j axis (trunk-2 evaluation points).  Each
of the 8 cores receives its 8 rows of x2, evaluates all MLPs (replicated,
~1% of the FLOPs), builds
    U^T[z, b*64+i] = bT[z,b] * t1T[z,i]                    (512 cols)
    W_p[z, (jl*64+k)*32+l] = t2T[z,jl]*t3T[z,k]*t4T[z,l]   (4x 4096 cols)
and evaluates its 512x16384 output shard as K=64 (zero-padded to 128)
bf16 matmuls (N=512), PSUM -> SBUF staging -> 2MB DMA stores.
"""

import numpy as np

DIM = 4
H = 256
R = 64
NF = 64
NCORES = 8
JS = 8                      # j points per core
N1, N2S, N3, N4 = 64, 8, 64, 32
NB = 8                      # branch batch
BI = NB * 64                # 512 output rows (b,i)
JKL = JS * 64 * 32          # 16384 output cols per core
MAGIC = 12582912.0          # 1.5 * 2**23 : fp32 round-to-nearest-int trick
TWO_PI = 6.283185307179586
INV_2PI = 1.0 / TWO_PI
HALF_PI = 1.5707963267948966

_CACHE = {}


def _build_nc(sim_compat=False):
    import concourse.mybir as mybir
    import concourse.tile as tile
    from concourse import bacc

    f32 = mybir.dt.float32
    f32r = mybir.dt.float32r  # noqa: F841  (kept for reference)
    bf16 = mybir.dt.float16  # fp16: same PE rate as bf16, 8x finer mantissa
    Alu = mybir.AluOpType
    Act = mybir.ActivationFunctionType

    nc = bacc.Bacc("TRN2", target_bir_lowering=False, debug=False,
                   num_devices=NCORES)

    def din(name, shape, dt=None):
        return nc.dram_tensor(name, list(shape), dt or f32,
                              kind="ExternalInput").ap()

    x_in = {1: din("x1", (1, N1)), 2: din("x2s", (1, N2S)),
            3: din("x3", (1, N3)), 4: din("x4", (1, N4))}
    smallpk = din("smallpk", (128, 40))
    tw0T = din("tw0T", (128, DIM, 1, H), bf16)
    tw12T = din("tw12T", (128, DIM, 4, H), bf16)
    tw3T = din("tw3T", (128, DIM, 2, R), bf16)
    fT = din("fT", (128, 2, NB), bf16)
    bw012T = din("bw012T", (128, 3, 2, H), bf16)
    bw3T = din("bw3T", (128, 2, R), bf16)

    out = nc.dram_tensor("out", [BI, JKL], f32, kind="ExternalOutput").ap()

    with tile.TileContext(nc) as tc, \
         tc.tile_pool(name="consts", bufs=1) as consts, \
         tc.tile_pool(name="mlp", bufs=3) as mlp, \
         tc.tile_pool(name="fact", bufs=1) as fact, \
         tc.tile_pool(name="stag", bufs=6) as stag, \
         tc.tile_pool(name="psum", bufs=4, space="PSUM") as psum:

        _ld = [0]

        def load(ap_dram, shape, name, dt=None):
            t = consts.tile(list(shape), dt or f32, tag=name, name=name)
            eng = nc.sync if _ld[0] % 2 == 0 else nc.scalar
            _ld[0] += 1
            eng.dma_start(t[:], ap_dram)
            return t

        sp = load(smallpk, (128, 40), "sp")
        tw0_sb = load(tw0T, (128, DIM, 1, H), "tw0", bf16)
        tw12_sb = load(tw12T, (128, DIM, 4, H), "tw12", bf16)
        tw3_sb = load(tw3T, (128, DIM, 2, R), "tw3", bf16)
        fT_sb = load(fT, (128, 2, NB), "fT", bf16)
        bw012_sb = load(bw012T, (128, 3, 2, H), "bw012", bf16)
        bw3_sb = load(bw3T, (128, 2, R), "bw3", bf16)

        # smallpk column map
        b2 = sp[:, 0:1]
        ph = sp[:, 1:2]
        tb = {(0, i): sp[:, 2 + 2 * i: 4 + 2 * i] for i in range(DIM)}
        tb.update({(1, i): sp[:, 10 + 2 * i: 12 + 2 * i] for i in range(DIM)})
        tb.update({(2, i): sp[:, 18 + 2 * i: 20 + 2 * i] for i in range(DIM)})
        bb = {k: sp[:, 26 + 2 * k: 28 + 2 * k] for k in range(3)}
        tb3 = {i: sp[0:R, 32 + i: 33 + i] for i in range(DIM)}
        bb3 = sp[0:R, 36:37]

        UT = fact.tile([128, BI], bf16, tag="UT", name="UT")
        nc.vector.memset(UT[:], 0.0)
        Wp = []
        for p in range(4):
            w = fact.tile([128, 2, 64, 32], bf16, tag=f"W{p}", name=f"W{p}")
            nc.gpsimd.memset(w[64:128], 0.0)
            Wp.append(w)

        def act_silu(out_chunk, ps, bias_col, n):
            if not sim_compat:
                nc.scalar.activation(out_chunk, ps, Act.Silu, bias=bias_col)
                return
            # CoreSim has no Silu: y*sigmoid(y), y = ps + bias
            y = mlp.tile([128, 64], f32, tag="sly", name="sly")[:ps.shape[0], :n]
            nc.vector.tensor_scalar(y, ps, bias_col, None, Alu.add)
            s = mlp.tile([128, 64], f32, tag="sls", name="sls")[:ps.shape[0], :n]
            nc.scalar.activation(s, y, Act.Sigmoid)
            nc.vector.tensor_mul(out_chunk, y, s)

        def dense(rhs_chunks, n, wsb, M, bias_ap2, out_chunks, act):
            # out = act(W @ rhs + b); wsb[:, ko, :] are the lhsT K-chunks
            KC = len(rhs_chunks)
            for mo in range((M + 127) // 128):
                mw = min(128, M - 128 * mo)
                ps = psum.tile([128, 1024], f32, tag="ps",
                               name="ps")[:mw, :n]
                for ko in range(KC):
                    nc.tensor.matmul(ps,
                                     lhsT=wsb[:, ko, 128 * mo:128 * mo + mw],
                                     rhs=rhs_chunks[ko],
                                     start=(ko == 0), stop=(ko == KC - 1))
                bias_col = bias_ap2[:mw, mo:mo + 1]
                if act == "silu":
                    act_silu(out_chunks[mo], ps, bias_col, n)
                else:
                    nc.scalar.activation(out_chunks[mo], ps,
                                         Act.Identity, bias=bias_col)

        def fourier(x_ap, n):
            # rows 0:64 cos(xB), rows 64:128 sin(xB) via range-reduced Sin
            xr = mlp.tile([128, 64], f32, tag="xr", name="xr")[:, :n]
            nc.sync.dma_start(xr, x_ap.to_broadcast((128, n)))
            q = mlp.tile([128, 64], f32, tag="q", name="q")[:, :n]
            nc.vector.tensor_scalar(q, xr, b2, ph, Alu.mult, Alu.add)
            u = mlp.tile([128, 64], f32, tag="u", name="u")[:, :n]
            nc.vector.tensor_scalar(u, q, INV_2PI, MAGIC, Alu.mult, Alu.add)
            nc.vector.tensor_scalar(u, u, MAGIC, TWO_PI, Alu.subtract,
                                    Alu.mult)
            nc.vector.tensor_sub(q, q, u)
            ft = mlp.tile([128, 64], bf16, tag="feat", name="feat")[:, :n]
            nc.scalar.activation(ft, q, Act.Sin)
            return ft

        def trunk(i, n):
            ft = fourier(x_in[i + 1], n)
            h1 = mlp.tile([128, 2, 64], bf16, tag="h1", name="h1")[:, :, :n]
            dense([ft], n, tw0_sb[:, i], H, tb[(0, i)],
                  [h1[:, 0], h1[:, 1]], "silu")
            h2 = mlp.tile([128, 2, 64], bf16, tag="h2", name="h2")[:, :, :n]
            dense([h1[:, 0], h1[:, 1]], n, tw12_sb[:, i, 0:2], H, tb[(1, i)],
                  [h2[:, 0], h2[:, 1]], "silu")
            h3 = mlp.tile([128, 2, 64], bf16, tag="h3", name="h3")[:, :, :n]
            dense([h2[:, 0], h2[:, 1]], n, tw12_sb[:, i, 2:4], H, tb[(2, i)],
                  [h3[:, 0], h3[:, 1]], "silu")
            t = fact.tile([R, n], f32, tag=f"t{i}", name=f"t{i}")
            tb3_2 = tb3[i][:, :]  # [64,1] -> bias_ap2 with one column
            dense([h3[:, 0], h3[:, 1]], n, tw3_sb[:, i], R, tb3_2, [t[:]],
                  "linear")
            return t

        # W-path trunks first so the Khatri-Rao operand is ready earliest
        t2T = trunk(1, N2S)
        t3T = trunk(2, N3)
        G = fact.tile([R, JS * 64], f32, tag="G", name="G")
        for jl in range(JS):
            nc.vector.tensor_scalar(G[:, 64 * jl:64 * (jl + 1)], t3T[:],
                                    t2T[:, jl:jl + 1], None, Alu.mult)
        t4T = trunk(3, N4)

        # branch MLP (no Fourier features)
        h1 = mlp.tile([128, 2, NB], bf16, tag="bh1", name="bh1")
        dense([fT_sb[:, 0], fT_sb[:, 1]], NB, bw012_sb[:, 0], H, bb[0],
              [h1[:, 0], h1[:, 1]], "silu")
        h2 = mlp.tile([128, 2, NB], bf16, tag="bh2", name="bh2")
        dense([h1[:, 0], h1[:, 1]], NB, bw012_sb[:, 1], H, bb[1],
              [h2[:, 0], h2[:, 1]], "silu")
        h3 = mlp.tile([128, 2, NB], bf16, tag="bh3", name="bh3")
        dense([h2[:, 0], h2[:, 1]], NB, bw012_sb[:, 2], H, bb[2],
              [h3[:, 0], h3[:, 1]], "silu")
        bT = fact.tile([R, NB], f32, tag="bT", name="bT")
        dense([h3[:, 0], h3[:, 1]], NB, bw3_sb, R, bb3[:, :], [bT[:]],
              "linear")

        t1T = trunk(0, N1)
        for b in range(NB):
            nc.vector.tensor_scalar(UT[0:R, 64 * b:64 * (b + 1)], t1T[:],
                                    bT[:, b:b + 1], None, Alu.mult)

        # main contraction, one jl-pair at a time so W gen overlaps matmuls
        Gv = G[:].rearrange("p (a b) -> p a b", a=JS)
        for p in range(4):
            w = Wp[p]
            for s in range(2):
                jl = 2 * p + s
                nc.vector.tensor_tensor(
                    w[0:R, s],
                    Gv[:, jl, :, None].to_broadcast((R, 64, 32)),
                    t4T[:, None, :].to_broadcast((R, 64, 32)),
                    Alu.mult)
            wf = w[:].rearrange("p a b c -> p (a b c)")
            for m in range(BI // 128):
                lhsT = UT[:, 128 * m:128 * (m + 1)]
                stg = stag.tile([128, 4096], f32, tag="stg", name="stg")
                for qi in range(4):
                    ps = psum.tile([128, 1024], f32, tag="ps", name="ps")
                    for h in range(2):
                        nc.tensor.matmul(
                            ps[:, 512 * h:512 * (h + 1)], lhsT=lhsT,
                            rhs=wf[:, 1024 * qi + 512 * h:
                                   1024 * qi + 512 * (h + 1)],
                            start=True, stop=True)
                    nc.any.tensor_copy(
                        stg[:, 1024 * qi:1024 * (qi + 1)], ps[:])
                (nc.sync if (p * 4 + m) % 2 == 0 else nc.scalar).dma_start(
                    out[128 * m:128 * (m + 1), 4096 * p:4096 * (p + 1)],
                    stg[:])

    nc.compile()
    return nc


def _prep_maps(inputs):
    def f32c(a):
        return np.ascontiguousarray(np.asarray(a), dtype=np.float32)

    i = {k: np.asarray(v) for k, v in inputs.items()}
    B = f32c(i["B"]).reshape(NF)

    sp = np.zeros((128, 40), np.float32)
    sp[:NF, 0] = B
    sp[NF:, 0] = B
    sp[:NF, 1] = HALF_PI        # cos rows get sin(x + pi/2)
    for k, key in enumerate(("tb0", "tb1", "tb2")):
        for t in range(DIM):
            sp[:, 2 + 8 * k + 2 * t: 4 + 8 * k + 2 * t] = \
                f32c(i[key][t]).reshape(2, 128).T
    for k, key in enumerate(("bb0", "bb1", "bb2")):
        sp[:, 26 + 2 * k: 28 + 2 * k] = f32c(i[key]).reshape(2, 128).T
    for t in range(DIM):
        sp[:R, 32 + t] = f32c(i["tb3"][t]).reshape(R)
    sp[:R, 36] = f32c(i["bb3"]).reshape(R)

    def wT(w, kc, m):            # (m, kc*128) -> (128, kc, m)
        return f32c(w).T.reshape(kc, 128, m).transpose(1, 0, 2)

    common = {
        "x1": f32c(i["x1"]).reshape(1, N1),
        "x3": f32c(i["x3"]).reshape(1, N3),
        "x4": f32c(i["x4"]).reshape(1, N4),
        "smallpk": sp,
        # (128, DIM, KC, M) stacks
        "tw0T": np.stack([wT(i["tW0"][t], 1, H) for t in range(DIM)], axis=1),
        "tw12T": np.stack(
            [np.concatenate([wT(i["tW1"][t], 2, H), wT(i["tW2"][t], 2, H)],
                            axis=1) for t in range(DIM)], axis=1),
        "tw3T": np.stack([wT(i["tW3"][t], 2, R) for t in range(DIM)], axis=1),
        "fT": f32c(i["f"]).T.reshape(2, 128, NB).transpose(1, 0, 2),
        "bw012T": np.stack([wT(i["bW0"], 2, H), wT(i["bW1"], 2, H),
                            wT(i["bW2"], 2, H)], axis=1),
        "bw3T": wT(i["bW3"], 2, R),
    }
    bf = ("tw0T", "tw12T", "tw3T", "fT", "bw012T", "bw3T")
    common = {k: np.ascontiguousarray(
                  v, dtype=(np.float16 if k in bf else np.float32))
              for k, v in common.items()}
    x2 = f32c(i["x2"]).reshape(64)
    maps = []
    for c in range(NCORES):
        m = dict(common)
        m["x2s"] = np.ascontiguousarray(
            x2[JS * c:JS * (c + 1)].reshape(1, JS))
        maps.append(m)
    return maps


def _gather(results):
    full = np.empty((NB, 64, 64, 64, 32, 1), np.float32)
    for c in range(NCORES):
        full[:, :, JS * c:JS * (c + 1), :, :, 0] = \
            results[c]["out"].reshape(NB, 64, JS, 64, 32)
    return full


def kernel(**inputs):
    from concourse.bass_utils import run_bass_kernel_spmd

    if "nc" not in _CACHE:
        _CACHE["nc"] = _build_nc()
    nc = _CACHE["nc"]
    maps = _prep_maps(inputs)
    res = run_bass_kernel_spmd(nc, maps, core_ids=list(range(NCORES)))
    return _gather(res.results)
